# revision 5
# baseline (speedup 1.0000x reference)
"""Capsule dynamic-routing kernel for Trainium2 (Bass/Tile), 8 NeuronCores.

Sharding: data-parallel over batch (B=64 -> 8 batches/core, grouped in 4
pairs of 2).  W (64x256) is tiny and folded into per-iteration stationary
operands; no collectives are needed (pure SPMD).

The reference computes
    u_hat = u @ W                      # (N, 256), col c = k*16+d
    b=0; for i in 3: c = softmax_k(b); s[k,:] = sum_n c[k,n]*u_hat[n,kblk];
         out = squash(s); b += <out, u_hat>
u_hat is (B,N,256) = 512 MiB and never fits on chip.  We never materialize
it.  Since b_i = <sum_{j<i} out_j, u_hat>, with O = accumulated outputs and
Obd its (256,16) block-diagonal expansion:
    b_i[k,n] = <Wo[:,k], u[n,:]>   where Wo = W @ Obd   (64x16, tiny)
    s[k,d]   = sum_e G[k,e] W[e,k*16+d],  G[k,e] = sum_n c[k,n] u[n,e]
so each routing iteration only streams u (SBUF-resident, bf16) through the
PE array.

Host/transfer optimization (v2): the host ships u ONCE, in natural layout,
as a zero-copy strided bf16 view of the f32 data (high uint16 halves ==
round-toward-zero bf16).  All layout work happens on device:
  * 16 DMAs (1 MiB each) land u with the free n-permutation
    n = h*64 + l  (h on partitions, l indexing 128-row "chunks"), free
    columns interleaved (l, b2, e) so the staging tile IS the n-on-
    partitions operand (un) of the G-pass.
  * 256 PE transposes build ut (e on partitions) for the b-pass.
  * iteration 0 (uniform softmax) collapses to a vector row-reduce of ut
    (colsum), skipping a whole G-pass.
The n-permutation is exact: softmax is per-n and every contraction sums
over n.
"""

import numpy as np
from contextlib import ExitStack

import ml_dtypes

import concourse.bass as bass
import concourse.bacc as bacc
import concourse.tile as tile
import concourse.mybir as mybir
from concourse.bass_utils import run_bass_kernel_spmd

dt = mybir.dt
AFT = mybir.ActivationFunctionType
AXT = mybir.AxisListType
ALU = mybir.AluOpType

B, N_FULL, D = 64, 8192, 64
K, DCAP, KD = 16, 16, 256
NCORES = 8
NB = 8            # batches per core
NP = 4            # batch pairs per core
ROUTINGS = 3
EPS = 1e-7
CHUNK = 128       # n per contraction chunk
SUP = 16          # chunks per softmax super-chunk
TG = 4            # chunks transposed per PSUM group

U_DT = dt.bfloat16
U_NP = ml_dtypes.bfloat16


def build_program(n=N_FULL, reps=1, ablate=()):
    assert n % 128 == 0
    nl = n // 128     # chunks (and rows per staging partition)
    nch = nl
    sup = min(SUP, nch)
    assert nch % sup == 0
    nsup = nch // sup
    assert nch % TG == 0
    ngrp = nch // TG
    f32 = dt.float32

    nc = bacc.Bacc("TRN2", target_bir_lowering=False, debug=False)

    u_d = nc.dram_tensor("u", [NB, 128, nl, D], U_DT, kind="ExternalInput").ap()
    wt_d = nc.dram_tensor("wt", [2, 128, D], U_DT, kind="ExternalInput").ap()
    wsb_d = nc.dram_tensor("wsb", [128, KD], U_DT, kind="ExternalInput").ap()
    mask_d = nc.dram_tensor("mask", [128, KD], f32, kind="ExternalInput").ap()
    ident_d = nc.dram_tensor("ident", [128, 128], f32, kind="ExternalInput").ap()
    identb_d = nc.dram_tensor("identb", [128, 128], U_DT, kind="ExternalInput").ap()
    out_d = nc.dram_tensor("out", [128, KD], f32, kind="ExternalOutput").ap()

    with tile.TileContext(nc) as tc, ExitStack() as ctx:
        consts = ctx.enter_context(tc.tile_pool(name="consts", bufs=1))
        resident = ctx.enter_context(tc.tile_pool(name="resident", bufs=1))
        work = ctx.enter_context(tc.tile_pool(name="work", bufs=1))
        c_pool = ctx.enter_context(tc.tile_pool(name="cpool", bufs=5))
        e_pool = ctx.enter_context(tc.tile_pool(name="epool", bufs=5))
        z_pool = ctx.enter_context(tc.tile_pool(name="zpool", bufs=8))
        ps_bb = ctx.enter_context(tc.tile_pool(name="psbb", bufs=2, space="PSUM"))
        ps_tp = ctx.enter_context(tc.tile_pool(name="pstp", bufs=2, space="PSUM"))
        ps_gt = ctx.enter_context(tc.tile_pool(name="psgt", bufs=1, space="PSUM"))

        # ---- constants ----
        wt_t = consts.tile([128, 2 * D], U_DT, tag="wt", name="wt")        # W.T halves
        for h in range(2):
            nc.sync.dma_start(out=wt_t[:, h * D:(h + 1) * D], in_=wt_d[h])
        wsb_t = consts.tile([128, KD], U_DT, tag="wsb", name="wsb")         # W stacked x2
        nc.sync.dma_start(out=wsb_t[:, :], in_=wsb_d[:, :])
        mask_t = consts.tile([128, KD], f32, tag="mask", name="mask")
        nc.sync.dma_start(out=mask_t[:, :], in_=mask_d[:, :])
        ident_t = consts.tile([128, 128], f32, tag="ident", name="ident")
        nc.sync.dma_start(out=ident_t[:, :], in_=ident_d[:, :])
        identb_t = consts.tile([128, 128], U_DT, tag="identb", name="identb")
        nc.sync.dma_start(out=identb_t[:, :], in_=identb_d[:, :])
        eps_t = consts.tile([128, 1], f32, tag="eps", name="eps")
        nc.vector.memset(eps_t[:, :], EPS)

        # ---- resident input copies ----
        # un[p]: partitions = h (n-high), free = (l, b2, e); col = l*128+b2*64+e
        # ut[p]: partitions = (b2, e),    free = (l, h);     col = l*128+h
        un_t = [resident.tile([128, n], U_DT, tag=f"un{p}", name=f"un{p}")
                for p in range(NP)]
        ut_t = [resident.tile([128, n], U_DT, tag=f"ut{p}", name=f"ut{p}")
                for p in range(NP)]

        def un_chunk(p, j):
            return un_t[p][:, j * CHUNK:(j + 1) * CHUNK]

        def ut_chunk(p, j):
            return ut_t[p][:, j * CHUNK:(j + 1) * CHUNK]

        # ---- persistent work tiles ----
        o_acc = work.tile([128, KD], f32, tag="oacc", name="oacc")      # masked output accum
        sm = work.tile([128, KD], f32, tag="sm", name="sm")
        sq = work.tile([128, KD], f32, tag="sq", name="sq")
        o_fin = work.tile([128, KD], f32, tag="ofin", name="ofin")
        t1_sb = work.tile([128, 128], U_DT, tag="t1", name="t1")      # Obd halves
        t2_sb = work.tile([128, 128], U_DT, tag="t2", name="t2")
        wop = [work.tile([128, 32], U_DT, tag=f"wop{p}", name=f"wop{p}") for p in range(NP)]
        gt_sb = [work.tile([128, 32], U_DT, tag=f"gts{p}", name=f"gts{p}") for p in range(NP)]
        s2 = work.tile([128, 1], f32, tag="s2", name="s2")
        cs_t = work.tile([128, NP], f32, tag="cs", name="cs")
        sc_a = work.tile([128, 1], f32, tag="sca", name="sca")
        sc_b = work.tile([128, 1], f32, tag="scb", name="scb")
        sc_c = work.tile([128, 1], f32, tag="scc", name="scc")
        sc_d = work.tile([128, 1], f32, tag="scd", name="scd")
        sc_e = work.tile([128, 1], f32, tag="sce", name="sce")

        gt_tiles = [ps_gt.tile([128, 32], f32, tag=f"gt{p}", name=f"gt{p}",
                       padded_shape=[128, 512]) for p in range(NP)]

        # cross-batch blocks of gt_sb / wop stay zero for the whole kernel
        for p in range(NP):
            nc.vector.memset(gt_sb[p][0:64, 16:32], 0.0)
            nc.vector.memset(gt_sb[p][64:128, 0:16], 0.0)
            nc.vector.memset(wop[p][0:64, 16:32], 0.0)
            nc.vector.memset(wop[p][64:128, 0:16], 0.0)

        # ---- input DMAs: natural-layout u -> un (b2-interleaved) ----
        if "nodma" not in ablate:
            for p in range(NP):
                for b2 in range(2):
                    dst = (un_t[p][:, :]
                           .rearrange("q (l x e) -> q l x e", x=2, e=D)
                           [:, :, b2, :])
                    nc.sync.dma_start(out=dst, in_=u_d[2 * p + b2])
        else:
            for p in range(NP):
                nc.vector.memset(un_t[p][:, 0:2], 0.0)

        # ---- PE transposes: un -> ut ----
        for p in range(NP):
            for g in range(ngrp):
                tp = ps_tp.tile([128, TG * 128], U_DT, tag="tp", name="tp",
                                padded_shape=[128, 1024])
                for q in range(TG):
                    j = g * TG + q
                    nc.tensor.transpose(tp[:, q * 128:(q + 1) * 128],
                                        un_chunk(p, j), identb_t[:, :])
                eng = nc.vector if (g % 2 == 0) else nc.scalar
                if eng is nc.vector:
                    eng.tensor_copy(ut_t[p][:, g * TG * 128:(g + 1) * TG * 128],
                                    tp[:, :])
                else:
                    eng.activation(ut_t[p][:, g * TG * 128:(g + 1) * TG * 128],
                                   tp[:, :], AFT.Copy)

        def routing_pass(it):
            """b-pass + softmax + G-pass, accumulating gt_tiles (it >= 1)."""
            for p in range(NP):
                for s in range(nsup):
                    bb = ps_bb.tile([128, sup * 32], f32, tag="bb", name="bb",
                                    padded_shape=[128, 512])
                    for rel in range(sup):
                        j = s * sup + rel
                        nc.tensor.matmul(
                            bb[:, rel * 32:(rel + 1) * 32],
                            lhsT=ut_chunk(p, j), rhs=wop[p][:, :],
                            start=(rel == 0), stop=(rel == sup - 1))
                    e_t = e_pool.tile([128, sup * 32], f32, tag="e", name="e")
                    nc.scalar.activation(e_t[:, :], bb[:, :], AFT.Exp)
                    z_t = z_pool.tile([128, sup * 2], f32, tag="z", name="z")
                    nc.vector.reduce_sum(
                        z_t[:, :].rearrange("p (a b) -> p a b", b=2),
                        e_t[:, :].rearrange("p (a b c) -> p a b c", b=2, c=K),
                        axis=AXT.X)
                    zr_t = z_pool.tile([128, sup * 2], f32, tag="zr", name="zr")
                    nc.vector.reciprocal(zr_t[:, :], z_t[:, :])
                    c_t = c_pool.tile([128, sup * 32], U_DT, tag="c", name="c")
                    nc.vector.tensor_mul(
                        c_t[:, :].rearrange("p (a b c) -> p a b c", b=2, c=K),
                        e_t[:, :].rearrange("p (a b c) -> p a b c", b=2, c=K),
                        zr_t[:, :].rearrange("p (a b) -> p a b", b=2)
                            .broadcast_to([128, sup, 2, K]))
                    for rel in range(sup):
                        j = s * sup + rel
                        nc.tensor.matmul(
                            gt_tiles[p][:, :],
                            lhsT=un_chunk(p, j),
                            rhs=c_t[:, rel * 32:(rel + 1) * 32],
                            start=(j == 0), stop=(j == nch - 1))

        def finalize(it):
            """gt -> s -> mask -> squash -> (o_acc | o_fin); update Wo."""
            if it == 0:
                # uniform c == 1/16: G[k,e] = colsum[e]/16 for every k
                for p in range(NP):
                    nc.vector.reduce_sum(cs_t[:, p:p + 1], ut_t[p][:, :],
                                         axis=AXT.X)
                for p in range(NP):
                    nc.vector.tensor_scalar_mul(
                        gt_sb[p][0:64, 0:16],
                        cs_t[0:64, p:p + 1].broadcast_to([64, K]), 1.0 / K)
                    nc.vector.tensor_scalar_mul(
                        gt_sb[p][64:128, 16:32],
                        cs_t[64:128, p:p + 1].broadcast_to([64, K]), 1.0 / K)
            else:
                # keep only the in-batch diagonal blocks of GT-pair;
                # cross-batch blocks are garbage and contract as zero
                for p in range(NP):
                    nc.vector.tensor_copy(gt_sb[p][0:64, 0:16],
                                          gt_tiles[p][0:64, 0:16])
                    nc.vector.tensor_copy(gt_sb[p][64:128, 16:32],
                                          gt_tiles[p][64:128, 16:32])
            for p in range(NP):
                sf = ps_bb.tile([32, KD], f32, tag="bb", name="sf",
                                padded_shape=[32, 512])
                nc.tensor.matmul(sf[:, :], lhsT=gt_sb[p][:, :],
                                 rhs=wsb_t[:, :], start=True, stop=True)
                # fused PSUM->SBUF copy + diagonal-block mask
                nc.vector.tensor_mul(sm[32 * p:32 * p + 32, :], sf[:, :],
                                     mask_t[32 * p:32 * p + 32, :])
            # squash: scale = s2/(1+s2)/sqrt(s2+EPS), s2 = sum_d sm^2 (row sum)
            nc.scalar.activation(sq[:, :], sm[:, :], AFT.Square,
                                 accum_out=s2[:, :])
            nc.vector.tensor_scalar_add(sc_a[:, :], s2[:, :], 1.0)
            nc.vector.reciprocal(sc_b[:, :], sc_a[:, :])
            nc.scalar.activation(sc_c[:, :], s2[:, :], AFT.Sqrt,
                                 bias=eps_t[:, :])
            nc.vector.reciprocal(sc_d[:, :], sc_c[:, :])
            nc.vector.tensor_mul(sc_e[:, :], sc_b[:, :], sc_d[:, :])
            nc.vector.tensor_mul(sc_e[:, :], sc_e[:, :], s2[:, :])
            tgt = o_fin if it == ROUTINGS - 1 else o_acc
            if it == 1:
                nc.vector.tensor_scalar_mul(o_fin[:, :], sm[:, :], sc_e[:, :])
                nc.vector.tensor_add(o_acc[:, :], o_acc[:, :], o_fin[:, :])
            else:
                nc.vector.tensor_scalar_mul(tgt[:, :], sm[:, :], sc_e[:, :])
            if it == ROUTINGS - 1:
                nc.sync.dma_start(out=out_d[:, :], in_=o_fin[:, :])
                return
            # Obd_b (256,16 block-diag of O_b) as columns of o_acc.T halves
            for h, t_sb in ((0, t1_sb), (1, t2_sb)):
                tp = ps_bb.tile([128, 128], f32, tag="bb", name="tpo",
                                padded_shape=[128, 512])
                nc.tensor.transpose(tp[:, :], o_acc[:, h * 128:(h + 1) * 128],
                                    ident_t[:, :])
                nc.vector.tensor_copy(t_sb[:, :], tp[:, :])
            # Wo_b = W @ Obd_b, accumulated over the two 128-row halves of W.T
            wo = ps_bb.tile([64, NB * K], f32, tag="bb", name="wo",
                            padded_shape=[64, 512])
            for h2 in range(2):
                for b in range(NB):
                    nc.tensor.matmul(
                        wo[:, b * K:(b + 1) * K],
                        lhsT=wt_t[:, h2 * D:(h2 + 1) * D],
                        rhs=(t1_sb, t2_sb)[h2][:, b * K:(b + 1) * K],
                        start=(h2 == 0 and b == 0),
                        stop=(h2 == 1 and b == NB - 1))
            for b in range(NB):
                p, h = b // 2, b % 2
                nc.vector.tensor_copy(
                    wop[p][64 * h:64 * h + 64, 16 * h:16 * h + 16],
                    wo[:, b * K:(b + 1) * K])

        for rep in range(reps):
            if "nocompute" not in ablate:
                finalize(0)
                for it in range(1, ROUTINGS):
                    routing_pass(it)
                    finalize(it)
            else:
                nc.vector.memset(o_fin[:, :], 0.0)
                nc.sync.dma_start(out=out_d[:, :], in_=o_fin[:, :])
            if rep < reps - 1:
                tc.strict_bb_all_engine_barrier()

    nc.compile()
    return nc


def host_inputs(u_shard, W):
    """Per-core DRAM inputs from an (NB, n, 64) f32 batch shard.

    Zero-copy: the high uint16 half of each f32 IS the round-toward-zero
    bf16 value; the (n) axis splits into (h=128, l=n//128) as a view.
    """
    nb, n, d = u_shard.shape
    assert d == D and u_shard.dtype == np.float32
    if not u_shard.flags.c_contiguous:
        u_shard = np.ascontiguousarray(u_shard)
    hb = (u_shard.view(np.uint16)[..., 1::2]
          .view(U_NP)
          .reshape(nb, 128, n // 128, D))
    return {"u": hb}


def host_consts(W):
    Wf = np.asarray(W, np.float32)
    wt = np.ascontiguousarray(Wf.T.reshape(2, 128, D)).astype(U_NP)
    wsb = np.ascontiguousarray(np.concatenate([Wf, Wf], 0)).astype(U_NP)
    base = np.kron(np.eye(K, dtype=np.float32), np.ones((1, DCAP), np.float32))
    mask = np.ascontiguousarray(np.tile(base, (NB, 1)))
    ident = np.eye(128, dtype=np.float32)
    identb = ident.astype(U_NP)
    return {"wt": wt, "wsb": wsb, "mask": mask, "ident": ident,
            "identb": identb}


def extract_output(res_out):
    """(128, 256) masked f32 -> (8, 16, 16) squashed capsule outputs."""
    ar = np.arange(K)
    return res_out.reshape(NB, K, K, DCAP)[:, ar, ar, :]


_PROG_CACHE = {}


def _get_prog(n=N_FULL, reps=1):
    key = (n, reps)
    if key not in _PROG_CACHE:
        _PROG_CACHE[key] = build_program(n, reps)
    return _PROG_CACHE[key]


def kernel(u_vecs, W):
    u = np.asarray(u_vecs, np.float32)
    assert u.shape == (B, N_FULL, D)
    if not u.flags.c_contiguous:
        u = np.ascontiguousarray(u)
    nc = _get_prog()
    consts = host_consts(W)
    in_maps = [dict(consts, **host_inputs(u[c * NB:(c + 1) * NB], W))
               for c in range(NCORES)]
    res = run_bass_kernel_spmd(nc, in_maps, core_ids=list(range(NCORES)))
    return np.concatenate(
        [extract_output(res.results[c]["out"]) for c in range(NCORES)], axis=0
    ).astype(np.float32)


# revision 15
# speedup vs baseline: 1.0059x; 1.0059x over previous
"""Capsule dynamic-routing kernel for Trainium2 (Bass/Tile), 8 NeuronCores.

Sharding: data-parallel over batch (B=64 -> 8 batches/core, grouped in 4
pairs of 2).  W (64x256) is tiny and folded into per-iteration stationary
operands; no collectives are needed (pure SPMD).

The reference computes
    u_hat = u @ W                      # (N, 256), col c = k*16+d
    b=0; for i in 3: c = softmax_k(b); s[k,:] = sum_n c[k,n]*u_hat[n,kblk];
         out = squash(s); b += <out, u_hat>
u_hat is (B,N,256) = 512 MiB and never fits on chip.  We never materialize
it.  Since b_i = <sum_{j<i} out_j, u_hat>, with O = accumulated outputs and
Obd its (256,16) block-diagonal expansion:
    b_i[k,n] = <Wo[:,k], u[n,:]>   where Wo = W @ Obd   (64x16, tiny)
    s[k,d]   = sum_e G[k,e] W[e,k*16+d],  G[k,e] = sum_n c[k,n] u[n,e]
so each routing iteration only streams u (SBUF-resident, bf16) through the
PE array.

Host/transfer optimization (v2): the host ships u ONCE, in natural layout,
as a zero-copy strided bf16 view of the f32 data (high uint16 halves ==
round-toward-zero bf16).  All layout work happens on device:
  * 16 DMAs (1 MiB each) land u with the free n-permutation
    n = h*64 + l  (h on partitions, l indexing 128-row "chunks"), free
    columns interleaved (l, b2, e) so the staging tile IS the n-on-
    partitions operand (un) of the G-pass.
  * 256 PE transposes build ut (e on partitions) for the b-pass.
  * iteration 0 (uniform softmax) collapses to a vector row-reduce of ut
    (colsum), skipping a whole G-pass.
The n-permutation is exact: softmax is per-n and every contraction sums
over n.
"""

import numpy as np
from contextlib import ExitStack

import ml_dtypes

import concourse.bass as bass
import concourse.bacc as bacc
import concourse.tile as tile
import concourse.mybir as mybir
from concourse.bass_utils import run_bass_kernel_spmd

dt = mybir.dt
AFT = mybir.ActivationFunctionType
AXT = mybir.AxisListType
ALU = mybir.AluOpType

B, N_FULL, D = 64, 8192, 64
K, DCAP, KD = 16, 16, 256
NCORES = 8
NB = 8            # batches per core
NP = 4            # batch pairs per core
ROUTINGS = 3
EPS = 1e-7
CHUNK = 128       # n per contraction chunk
SUP = 16          # chunks per softmax super-chunk
TG = 4            # chunks transposed per PSUM group

U_DT = dt.bfloat16
U_NP = ml_dtypes.bfloat16


def build_program(n=N_FULL, reps=1, ablate=()):
    assert n % 128 == 0
    nl = n // 128     # chunks (and rows per staging partition)
    nch = nl
    sup = min(SUP, nch)
    assert nch % sup == 0
    nsup = nch // sup
    assert nch % TG == 0
    ngrp = nch // TG
    f32 = dt.float32

    nc = bacc.Bacc("TRN2", target_bir_lowering=False, debug=False)

    u_d = nc.dram_tensor("u", [NB, 128, nl, D], U_DT, kind="ExternalInput").ap()
    wt_d = nc.dram_tensor("wt", [2, 128, D], U_DT, kind="ExternalInput").ap()
    wsb_d = nc.dram_tensor("wsb", [128, KD], U_DT, kind="ExternalInput").ap()
    out_d = nc.dram_tensor("out", [128, KD], U_DT, kind="ExternalOutput").ap()

    with tile.TileContext(nc) as tc, ExitStack() as ctx:
        consts = ctx.enter_context(tc.tile_pool(name="consts", bufs=1))
        resident = ctx.enter_context(tc.tile_pool(name="resident", bufs=1))
        work = ctx.enter_context(tc.tile_pool(name="work", bufs=1))
        c_pool = ctx.enter_context(tc.tile_pool(name="cpool", bufs=5))
        e_pool = ctx.enter_context(tc.tile_pool(name="epool", bufs=5))
        z_pool = ctx.enter_context(tc.tile_pool(name="zpool", bufs=8))
        ps_bb = ctx.enter_context(tc.tile_pool(name="psbb", bufs=2, space="PSUM"))
        ps_tp = ctx.enter_context(tc.tile_pool(name="pstp", bufs=2, space="PSUM"))
        ps_gt = ctx.enter_context(tc.tile_pool(name="psgt", bufs=1, space="PSUM"))

        # ---- constants ----
        wt_t = consts.tile([128, 2 * D], U_DT, tag="wt", name="wt")        # W.T halves
        for h in range(2):
            nc.sync.dma_start(out=wt_t[:, h * D:(h + 1) * D], in_=wt_d[h])
        wsb_t = consts.tile([128, KD], U_DT, tag="wsb", name="wsb")         # W stacked x2
        nc.sync.dma_start(out=wsb_t[:, :], in_=wsb_d[:, :])
        mask_t = consts.tile([128, KD], f32, tag="mask", name="mask")
        ident_t = consts.tile([128, 128], f32, tag="ident", name="ident")
        identb_t = consts.tile([128, 128], U_DT, tag="identb", name="identb")
        scr_t = consts.tile([128, KD], f32, tag="scr", name="scr")
        kr_t = consts.tile([128, 1], f32, tag="kr", name="kr")
        # mask[r, (k2,d)] = (k2 == r % 16); ident[p, j] = (j == p)
        nc.gpsimd.iota(scr_t[:, :].rearrange("p (a b) -> p a b", b=DCAP),
                       [[1, K], [0, DCAP]], channel_multiplier=0,
                       allow_small_or_imprecise_dtypes=True)
        kri_t = consts.tile([128, 1], dt.int32, tag="kri", name="kri")
        nc.gpsimd.iota(kri_t[:, :], [[0, 1]], channel_multiplier=1)
        nc.vector.tensor_scalar(kri_t[:, :], kri_t[:, :], K - 1, None,
                                ALU.bitwise_and)
        nc.vector.tensor_copy(kr_t[:, :], kri_t[:, :])
        # mask = max(0, 1 - |k2 - (r & 15)|)
        nc.vector.tensor_scalar_sub(scr_t[:, :], scr_t[:, :], kr_t[:, 0:1])
        nc.scalar.activation(scr_t[:, :], scr_t[:, :], AFT.Abs)
        nc.vector.tensor_scalar(mask_t[:, :], scr_t[:, :], -1.0, 1.0,
                                ALU.mult, ALU.add)
        nc.vector.tensor_scalar_max(mask_t[:, :], mask_t[:, :], 0.0)
        # ident = max(0, 1 - |j - p|)
        nc.gpsimd.iota(scr_t[:, 0:128], [[1, 128]], channel_multiplier=-1,
                       allow_small_or_imprecise_dtypes=True)
        nc.scalar.activation(scr_t[:, 0:128], scr_t[:, 0:128], AFT.Abs)
        nc.vector.tensor_scalar(ident_t[:, :], scr_t[:, 0:128], -1.0, 1.0,
                                ALU.mult, ALU.add)
        nc.vector.tensor_scalar_max(ident_t[:, :], ident_t[:, :], 0.0)
        nc.vector.tensor_copy(identb_t[:, :], ident_t[:, :])
        eps_t = consts.tile([128, 1], f32, tag="eps", name="eps")
        nc.vector.memset(eps_t[:, :], EPS)

        # ---- resident input copies ----
        # un[p]: partitions = h (n-high), free = (l, b2, e); col = l*128+b2*64+e
        # ut[p]: partitions = (b2, e),    free = (l, h);     col = l*128+h
        un_t = [resident.tile([128, n], U_DT, tag=f"un{p}", name=f"un{p}")
                for p in range(NP)]
        ut_t = [resident.tile([128, n], U_DT, tag=f"ut{p}", name=f"ut{p}")
                for p in range(NP)]

        def un_chunk(p, j):
            return un_t[p][:, j * CHUNK:(j + 1) * CHUNK]

        def ut_chunk(p, j):
            return ut_t[p][:, j * CHUNK:(j + 1) * CHUNK]

        # ---- persistent work tiles ----
        o_acc = work.tile([128, KD], f32, tag="oacc", name="oacc")      # masked output accum
        sm = work.tile([128, KD], f32, tag="sm", name="sm")
        sq = work.tile([128, KD], f32, tag="sq", name="sq")
        o_fin = work.tile([128, KD], f32, tag="ofin", name="ofin")
        ob_t = work.tile([128, KD], U_DT, tag="ob", name="ob")
        t1_sb = work.tile([128, 128], U_DT, tag="t1", name="t1")      # Obd halves
        t2_sb = work.tile([128, 128], U_DT, tag="t2", name="t2")
        wop = [work.tile([128, 32], U_DT, tag=f"wop{p}", name=f"wop{p}") for p in range(NP)]
        gt_sb = [work.tile([128, 32], U_DT, tag=f"gts{p}", name=f"gts{p}") for p in range(NP)]
        s2 = work.tile([128, 1], f32, tag="s2", name="s2")
        cs_t = work.tile([128, NP], f32, tag="cs", name="cs")
        sc_a = work.tile([128, 1], f32, tag="sca", name="sca")
        sc_b = work.tile([128, 1], f32, tag="scb", name="scb")
        sc_c = work.tile([128, 1], f32, tag="scc", name="scc")
        sc_d = work.tile([128, 1], f32, tag="scd", name="scd")
        sc_e = work.tile([128, 1], f32, tag="sce", name="sce")

        gt_tiles = [ps_gt.tile([128, 32], f32, tag=f"gt{p}", name=f"gt{p}",
                       padded_shape=[128, 512]) for p in range(NP)]

        # cross-batch blocks of gt_sb / wop stay zero for the whole kernel
        for p in range(NP):
            nc.vector.memset(gt_sb[p][0:64, 16:32], 0.0)
            nc.vector.memset(gt_sb[p][64:128, 0:16], 0.0)
            nc.vector.memset(wop[p][0:64, 16:32], 0.0)
            nc.vector.memset(wop[p][64:128, 0:16], 0.0)

        # ---- input DMAs: natural-layout u -> un (b2-interleaved) ----
        if "nodma" not in ablate:
            for p in range(NP):
                for b2 in range(2):
                    dst = (un_t[p][:, :]
                           .rearrange("q (l x e) -> q l x e", x=2, e=D)
                           [:, :, b2, :])
                    nc.sync.dma_start(out=dst, in_=u_d[2 * p + b2])
        else:
            for p in range(NP):
                nc.vector.memset(un_t[p][:, 0:2], 0.0)

        # ---- PE transposes: un -> ut ----
        for p in range(NP):
            for g in range(ngrp):
                tp = ps_tp.tile([128, TG * 128], U_DT, tag="tp", name="tp",
                                padded_shape=[128, 1024])
                for q in range(TG):
                    j = g * TG + q
                    nc.tensor.transpose(tp[:, q * 128:(q + 1) * 128],
                                        un_chunk(p, j), identb_t[:, :])
                eng = nc.vector if (g % 2 == 0) else nc.scalar
                if eng is nc.vector:
                    eng.tensor_copy(ut_t[p][:, g * TG * 128:(g + 1) * TG * 128],
                                    tp[:, :])
                else:
                    eng.activation(ut_t[p][:, g * TG * 128:(g + 1) * TG * 128],
                                   tp[:, :], AFT.Copy)

        def routing_pass(it):
            """b-pass + softmax + G-pass, accumulating gt_tiles (it >= 1)."""
            for p in range(NP):
                for s in range(nsup):
                    bb = ps_bb.tile([128, sup * 32], f32, tag="bb", name="bb",
                                    padded_shape=[128, 512])
                    for rel in range(sup):
                        j = s * sup + rel
                        nc.tensor.matmul(
                            bb[:, rel * 32:(rel + 1) * 32],
                            lhsT=ut_chunk(p, j), rhs=wop[p][:, :],
                            start=(rel == 0), stop=(rel == sup - 1))
                    e_t = e_pool.tile([128, sup * 32], f32, tag="e", name="e")
                    nc.scalar.activation(e_t[:, :], bb[:, :], AFT.Exp)
                    z_t = z_pool.tile([128, sup * 2], f32, tag="z", name="z")
                    nc.vector.reduce_sum(
                        z_t[:, :].rearrange("p (a b) -> p a b", b=2),
                        e_t[:, :].rearrange("p (a b c) -> p a b c", b=2, c=K),
                        axis=AXT.X)
                    zr_t = z_pool.tile([128, sup * 2], f32, tag="zr", name="zr")
                    nc.vector.reciprocal(zr_t[:, :], z_t[:, :])
                    c_t = c_pool.tile([128, sup * 32], U_DT, tag="c", name="c")
                    nc.vector.tensor_mul(
                        c_t[:, :].rearrange("p (a b c) -> p a b c", b=2, c=K),
                        e_t[:, :].rearrange("p (a b c) -> p a b c", b=2, c=K),
                        zr_t[:, :].rearrange("p (a b) -> p a b", b=2)
                            .broadcast_to([128, sup, 2, K]))
                    for rel in range(sup):
                        j = s * sup + rel
                        nc.tensor.matmul(
                            gt_tiles[p][:, :],
                            lhsT=un_chunk(p, j),
                            rhs=c_t[:, rel * 32:(rel + 1) * 32],
                            start=(j == 0), stop=(j == nch - 1))

        def finalize(it):
            """gt -> s -> mask -> squash -> (o_acc | o_fin); update Wo."""
            if it == 0:
                # uniform c == 1/16: G[k,e] = colsum[e]/16 for every k
                for p in range(NP):
                    nc.vector.reduce_sum(cs_t[:, p:p + 1], ut_t[p][:, :],
                                         axis=AXT.X)
                for p in range(NP):
                    nc.vector.tensor_scalar_mul(
                        gt_sb[p][0:64, 0:16],
                        cs_t[0:64, p:p + 1].broadcast_to([64, K]), 1.0 / K)
                    nc.vector.tensor_scalar_mul(
                        gt_sb[p][64:128, 16:32],
                        cs_t[64:128, p:p + 1].broadcast_to([64, K]), 1.0 / K)
            else:
                # keep only the in-batch diagonal blocks of GT-pair;
                # cross-batch blocks are garbage and contract as zero
                for p in range(NP):
                    nc.vector.tensor_copy(gt_sb[p][0:64, 0:16],
                                          gt_tiles[p][0:64, 0:16])
                    nc.vector.tensor_copy(gt_sb[p][64:128, 16:32],
                                          gt_tiles[p][64:128, 16:32])
            for p in range(NP):
                sf = ps_bb.tile([32, KD], f32, tag="bb", name="sf",
                                padded_shape=[32, 512])
                nc.tensor.matmul(sf[:, :], lhsT=gt_sb[p][:, :],
                                 rhs=wsb_t[:, :], start=True, stop=True)
                # fused PSUM->SBUF copy + diagonal-block mask
                nc.vector.tensor_mul(sm[32 * p:32 * p + 32, :], sf[:, :],
                                     mask_t[32 * p:32 * p + 32, :])
            # squash: scale = s2/(1+s2)/sqrt(s2+EPS), s2 = sum_d sm^2 (row sum)
            nc.scalar.activation(sq[:, :], sm[:, :], AFT.Square,
                                 accum_out=s2[:, :])
            nc.vector.tensor_scalar_add(sc_a[:, :], s2[:, :], 1.0)
            nc.vector.reciprocal(sc_b[:, :], sc_a[:, :])
            nc.scalar.activation(sc_c[:, :], s2[:, :], AFT.Sqrt,
                                 bias=eps_t[:, :])
            nc.vector.reciprocal(sc_d[:, :], sc_c[:, :])
            nc.vector.tensor_mul(sc_e[:, :], sc_b[:, :], sc_d[:, :])
            nc.vector.tensor_mul(sc_e[:, :], sc_e[:, :], s2[:, :])
            if it == ROUTINGS - 1:
                nc.vector.tensor_scalar_mul(ob_t[:, :], sm[:, :], sc_e[:, :])
                nc.sync.dma_start(out=out_d[:, :], in_=ob_t[:, :])
                return
            if it == 1:
                nc.vector.tensor_scalar_mul(o_fin[:, :], sm[:, :], sc_e[:, :])
                nc.vector.tensor_add(o_acc[:, :], o_acc[:, :], o_fin[:, :])
            else:
                nc.vector.tensor_scalar_mul(o_acc[:, :], sm[:, :], sc_e[:, :])
            # Obd_b (256,16 block-diag of O_b) as columns of o_acc.T halves
            for h, t_sb in ((0, t1_sb), (1, t2_sb)):
                tp = ps_bb.tile([128, 128], f32, tag="bb", name="tpo",
                                padded_shape=[128, 512])
                nc.tensor.transpose(tp[:, :], o_acc[:, h * 128:(h + 1) * 128],
                                    ident_t[:, :])
                nc.vector.tensor_copy(t_sb[:, :], tp[:, :])
            # Wo_b = W @ Obd_b, accumulated over the two 128-row halves of W.T
            wo = ps_bb.tile([64, NB * K], f32, tag="bb", name="wo",
                            padded_shape=[64, 512])
            for h2 in range(2):
                for b in range(NB):
                    nc.tensor.matmul(
                        wo[:, b * K:(b + 1) * K],
                        lhsT=wt_t[:, h2 * D:(h2 + 1) * D],
                        rhs=(t1_sb, t2_sb)[h2][:, b * K:(b + 1) * K],
                        start=(h2 == 0 and b == 0),
                        stop=(h2 == 1 and b == NB - 1))
            for b in range(NB):
                p, h = b // 2, b % 2
                nc.vector.tensor_copy(
                    wop[p][64 * h:64 * h + 64, 16 * h:16 * h + 16],
                    wo[:, b * K:(b + 1) * K])

        for rep in range(reps):
            if "nocompute" not in ablate:
                finalize(0)
                for it in range(1, ROUTINGS):
                    routing_pass(it)
                    finalize(it)
            else:
                nc.vector.memset(ob_t[:, :], 0.0)
                nc.sync.dma_start(out=out_d[:, :], in_=ob_t[:, :])
            if rep < reps - 1:
                tc.strict_bb_all_engine_barrier()

    nc.compile()
    return nc


def host_inputs(u_shard, W):
    """Per-core DRAM inputs from an (NB, n, 64) f32 batch shard.

    Zero-copy: the high uint16 half of each f32 IS the round-toward-zero
    bf16 value; the (n) axis splits into (h=128, l=n//128) as a view.
    """
    nb, n, d = u_shard.shape
    assert d == D and u_shard.dtype == np.float32
    if not u_shard.flags.c_contiguous:
        u_shard = np.ascontiguousarray(u_shard)
    hb = (u_shard.view(np.uint16)[..., 1::2]
          .view(U_NP)
          .reshape(nb, 128, n // 128, D))
    return {"u": hb}


def host_consts(W):
    Wf = np.asarray(W, np.float32)
    wt = np.ascontiguousarray(Wf.T.reshape(2, 128, D)).astype(U_NP)
    wsb = np.ascontiguousarray(np.concatenate([Wf, Wf], 0)).astype(U_NP)
    return {"wt": wt, "wsb": wsb}


def extract_output(res_out):
    """(128, 256) masked bf16 -> (8, 16, 16) squashed capsule outputs."""
    ar = np.arange(K)
    return np.asarray(res_out).astype(np.float32).reshape(
        NB, K, K, DCAP)[:, ar, ar, :]


# ---------------------------------------------------------------------------
# bass2jax.run_bass_via_pjrt builds a fresh jax.jit wrapper on every call,
# forcing a full re-trace per kernel() invocation (~100-300 ms of pure host
# overhead).  Memoize the traced executable per Bass program; semantics are
# identical (same primitive bind, same donation), and any failure falls back
# to the original implementation.
import concourse.bass2jax as _b2j

_ORIG_RUN_VIA_PJRT = _b2j.run_bass_via_pjrt
_PJRT_PLAN_CACHE = {}
_CONCAT_BUF_CACHE = {}


def _pjrt_plan(nc, n_cores):
    import jax
    from jax.sharding import Mesh, PartitionSpec
    from jax.experimental.shard_map import shard_map

    _b2j.install_neuronx_cc_hook()
    in_names, out_names, out_avals, out_shapes = [], [], [], []
    partition_name = (nc.partition_id_tensor.name
                      if nc.partition_id_tensor else None)
    for alloc in nc.m.functions[0].allocations:
        if not isinstance(alloc, mybir.MemoryLocationSet):
            continue
        name = alloc.memorylocations[0].name
        if alloc.kind == "ExternalInput":
            if name != partition_name:
                in_names.append(name)
        elif alloc.kind == "ExternalOutput":
            out_names.append(name)
            shape = tuple(alloc.tensor_shape)
            dtype = mybir.dt.np(alloc.dtype)
            out_avals.append(jax.core.ShapedArray(shape, dtype))
            out_shapes.append((shape, dtype))
    n_params, n_outs = len(in_names), len(out_avals)
    all_names = tuple(in_names + out_names +
                      ([partition_name] if partition_name else []))

    def _body(*args):
        operands = list(args)
        if partition_name is not None:
            operands.append(_b2j.partition_id_tensor())
        return tuple(_b2j._bass_exec_p.bind(
            *operands, out_avals=tuple(out_avals), in_names=all_names,
            out_names=tuple(out_names), lowering_input_output_aliases=(),
            sim_require_finite=True, sim_require_nnan=True, nc=nc))

    devices = jax.devices()[:n_cores]
    assert len(devices) == n_cores
    mesh = Mesh(np.asarray(devices), ("core",))
    in_specs = (PartitionSpec("core"),) * (n_params + n_outs)
    out_specs = (PartitionSpec("core"),) * n_outs
    donate = tuple(range(n_params, n_params + n_outs))
    sharded = jax.jit(
        shard_map(_body, mesh=mesh, in_specs=in_specs, out_specs=out_specs,
                  check_rep=False),
        donate_argnums=donate, keep_unused=True)
    return (sharded, in_names, out_names, out_shapes, n_params)


def _cached_run_via_pjrt(nc, in_maps, n_cores):
    try:
        if n_cores < 2 or getattr(nc, "dbg_addr", None) is not None:
            return _ORIG_RUN_VIA_PJRT(nc, in_maps, n_cores)
        key = (id(nc), n_cores)
        plan = _PJRT_PLAN_CACHE.get(key)
        if plan is None:
            plan = _pjrt_plan(nc, n_cores)
            _PJRT_PLAN_CACHE[key] = plan
        sharded, in_names, out_names, out_shapes, n_params = plan
        # gather per-core arrays into cached preallocated buffers (avoids
        # 67 MB of fresh-page faults per call vs np.concatenate)
        bufs = _CONCAT_BUF_CACHE.get(key)
        if bufs is None:
            bufs = {}
            for name in in_names:
                parts = [np.asarray(m[name]) for m in in_maps]
                bufs[name] = np.empty(
                    (sum(p.shape[0] for p in parts), *parts[0].shape[1:]),
                    parts[0].dtype)
            _CONCAT_BUF_CACHE[key] = bufs
        concat_in = []
        for name in in_names:
            buf = bufs[name]
            off = 0
            for m in in_maps:
                a = np.asarray(m[name])
                np.copyto(buf[off:off + a.shape[0]], a)
                off += a.shape[0]
            concat_in.append(buf)
        concat_zeros = [
            np.zeros((n_cores * s[0], *s[1:]), dtp) for s, dtp in out_shapes]
        out_arrs = sharded(*concat_in, *concat_zeros)
        return [
            {name: np.asarray(out_arrs[i]).reshape(
                n_cores, *out_shapes[i][0])[c]
             for i, name in enumerate(out_names)}
            for c in range(n_cores)]
    except Exception:
        return _ORIG_RUN_VIA_PJRT(nc, in_maps, n_cores)


_b2j.run_bass_via_pjrt = _cached_run_via_pjrt

_PROG_CACHE = {}


def _get_prog(n=N_FULL, reps=1):
    key = (n, reps)
    if key not in _PROG_CACHE:
        _PROG_CACHE[key] = build_program(n, reps)
    return _PROG_CACHE[key]


def kernel(u_vecs, W):
    u = np.asarray(u_vecs, np.float32)
    assert u.shape == (B, N_FULL, D)
    if not u.flags.c_contiguous:
        u = np.ascontiguousarray(u)
    nc = _get_prog()
    consts = host_consts(W)
    in_maps = [dict(consts, **host_inputs(u[c * NB:(c + 1) * NB], W))
               for c in range(NCORES)]
    res = run_bass_kernel_spmd(nc, in_maps, core_ids=list(range(NCORES)))
    return np.concatenate(
        [extract_output(res.results[c]["out"]) for c in range(NCORES)], axis=0
    ).astype(np.float32)
